# revision 33
# baseline (speedup 1.0000x reference)
"""Trainium2 Bass kernel for nn_MultiHeadSelfAttention (N=2, S=2048, E=1024, H=16).

Sharding: heads+batch tensor-parallel over 8 cores. Core c handles batch
n = c // 4 and 4 heads h in [4*(c%4), 4*(c%4)+4). Heads are processed in
PAIRS stacked on SBUF partition halves so the PE runs in split (64x128)
row-tiled mode for the whole attention phase:

  energy:  T0 (rows 0-63, head A dims) and T8 (rows 64-127, head B dims)
           execute CONCURRENTLY on the PE array quadrant halves, writing
           separate PSUM banks of one [128, 1024] tile -> ~2x energy rate.
  exp:     ScalarE/VectorE alternate whole [128, 1024] chunks (9:7).
  attV:    per key-chunk, 4 half-contraction matmuls ordered
           [A_T0 || B_T8], [B_T0 || A_T8] so row halves overlap pairwise
           with no same-bank concurrency and no tiling-mode switch; each
           head's o accumulates hi+lo halves sequentially in its own bank.
           The V ones-column yields the softmax denominator as row 64.
  norm:    den row DMA-broadcast to 64 partitions, X = o / den (DVE divide).
  fc_out:  row-parallel, head-pair stacked (contraction 128): y partial
           over this core's 4 heads; host sums 4 partials + bias.
"""

import time

import numpy as np

N_CORES = 8
NB = 2          # batch
S = 2048        # sequence length
E = 1024        # embed size
H = 16          # heads
D = 64          # head dim
HPC = 4         # heads per core
NPAIR = 2       # head pairs per core
SCALE = float(1.0 / np.sqrt(E))  # softmax scale (embed_size**0.5)

KC = S // 128   # 16 contraction chunks of 128 keys
QB = S // 512   # 4 query blocks of 512

# exp(x) ~= p(t)^8, p monic cubic, t = x pre-scaled by S_Q on the host.
# ScalarE chunks use exp(t * ACT_SCALE) (exact); VectorE chunks use the
# 8-stage custom DVE polynomial (rel err ~2.6e-4).
ALPHA = 6.0 ** (1.0 / 3.0)
S_Q = float(SCALE / (8.0 * ALPHA))   # host pre-scale on Q (t = S_Q/SCALE * x)
ACT_SCALE = float(8.0 * ALPHA)
B2, B1, B0 = 1.6574587989430332, 1.8171403999384372, 0.9999891634709047

# groups (key chunks) within a (pair, qb) unit whose exp runs on the DVE
DVE_GROUPS = frozenset({2, 4, 6, 9, 11, 13})


def register_exp_op():
    """Register the EXP_POLY8_ANT custom DVE op (idempotent)."""
    import concourse.dve_ops as dve_ops
    from concourse.dve_ops import OPS, DveOp
    from concourse.dve_spec import C0, C1, C2, Spec, Src0, _has_src1, lower, sq
    from concourse.dve_uop import DveOpSpec

    name = "EXP_POLY8_ANT"
    for op in OPS:
        if op.name == name:
            return op

    _p = ((Src0 + C0) * Src0 + C1) * Src0 + C2
    _body = sq(sq(sq(_p)))

    def _ref(in0, in1, s0, s1, imm2):
        p = ((in0 + s0) * in0 + s1) * in0 + imm2
        return ((p ** 2) ** 2) ** 2

    spec = Spec(body=_body, reference=_ref)
    opcode = dve_ops._CUSTOM_DVE_ROW_BASE + len(OPS)
    shas = {}
    for ver in ("v3", "v4"):
        d = DveOpSpec(
            name=name, opcode=opcode, uops=lower(spec, ver=ver),
            rd1_en=_has_src1(spec),
        )
        shas[ver] = d.sha(ver)
    op = DveOp(name, spec, subdim=False, uops_sha=shas)
    OPS.append(op)
    dve_ops._SUB_OPCODE_FOR_NAME[name] = opcode
    dve_ops.CUSTOM_DVE_SPECS[name] = spec
    return op


def build_kernel(reps=1, dve_groups=DVE_GROUPS, psum_recip=False, att_batch=2):
    import contextlib

    import concourse.bacc as bacc
    import concourse.bass as bass
    import concourse.tile as tile
    from concourse import mybir

    F32 = mybir.dt.float32
    F32R = mybir.dt.float32r
    BF16 = mybir.dt.bfloat16

    exp_op = register_exp_op()

    nc = bacc.Bacc("TRN2", target_bir_lowering=False, num_devices=N_CORES)

    # per-head inputs; K duplicated on both partition halves, Q's two 512-q
    # sub-blocks of each 1024-q unit stacked on partition halves, so the
    # row-tiled energy pair covers 1024 q's per chunk and attV/exp work on
    # one head x 1024 q's per instruction (bf16 moving max is 1024 free)
    qt = nc.dram_tensor("qt", [HPC, 128, S // 2], BF16, kind="ExternalInput")
    kt = nc.dram_tensor("kt", [HPC, 128, S], BF16, kind="ExternalInput")
    vb = nc.dram_tensor("vb", [HPC, S, D + 1], BF16, kind="ExternalInput")
    wt = nc.dram_tensor("wt", [NPAIR, 128, E], BF16, kind="ExternalInput")
    tok = nc.dram_tensor("tok", [1, 128], F32, kind="ExternalInput")
    yp = nc.dram_tensor("yp", [S, E], F32, kind="ExternalOutput")
    tok_out = nc.dram_tensor("tok_out", [1, 128], F32, kind="ExternalOutput")

    with tile.TileContext(nc) as tc:
        with contextlib.ExitStack() as ctx:
            singles = ctx.enter_context(tc.tile_pool(name="singles", bufs=1))
            vpool = ctx.enter_context(tc.tile_pool(name="vpool", bufs=4))
            epool = ctx.enter_context(
                tc.tile_pool(name="epool", bufs=2, space="PSUM")
            )
            opool = ctx.enter_context(
                tc.tile_pool(name="opool", bufs=2, space="PSUM")
            )
            apool = ctx.enter_context(tc.tile_pool(name="apool", bufs=5))
            npool = ctx.enter_context(tc.tile_pool(name="npool", bufs=2))
            ysb_pool = ctx.enter_context(tc.tile_pool(name="ysb", bufs=4))

            # token passthrough for timing chains
            tok_sb = singles.tile([1, 128], F32)
            nc.gpsimd.dma_start(out=tok_sb, in_=tok[:, :])
            nc.gpsimd.dma_start(out=tok_out[:, :], in_=tok_sb)

            # resident inputs (outside the reps loop)
            qt_sb, kt_sb, wt_sb, xt_sb = [], [], [], []
            for h in range(HPC):
                q_t = singles.tile([128, S // 2], BF16, tag=f"qt{h}")
                nc.sync.dma_start(out=q_t, in_=qt[h])
                qt_sb.append(q_t)
                k_t = singles.tile([128, S], BF16, tag=f"kt{h}")
                nc.sync.dma_start(out=k_t, in_=kt[h])
                kt_sb.append(k_t)
            for p in range(NPAIR):
                w_t = singles.tile([128, E], BF16, tag=f"wt{p}")
                nc.sync.dma_start(out=w_t, in_=wt[p])
                wt_sb.append(w_t)
                x_t = singles.tile([128, S], BF16, tag=f"xt{p}")
                xt_sb.append(x_t)

            loop_cm = tc.For_i(0, reps, 1) if reps > 1 else contextlib.nullcontext()
            ctx.enter_context(loop_cm)

            # V tiles: [128 keys-in-chunk, kc, 65], one per head, prefetched
            v_sb = []
            for h in range(HPC):
                v_t = vpool.tile([128, KC, D + 1], BF16, tag=f"v{h}")
                nc.sync.dma_start(
                    out=v_t, in_=vb[h].rearrange("(kc p) c -> p kc c", p=128)
                )
                v_sb.append(v_t)

            pend_att = None   # deferred attV emission (one group behind)
            pend_norm = None  # deferred normalization (one unit behind)

            def make_att(vH, a_t, o_t, k):
                def emit():
                    # full-row (128-contraction) attV; both halves share the
                    # same stationary V chunk (single weight load)
                    nc.tensor.matmul(
                        o_t[:, 0:512],
                        lhsT=vH[:, k, :],
                        rhs=a_t[:, 0:512],
                        start=(k == 0), stop=(k == KC - 1),
                    )
                    nc.tensor.matmul(
                        o_t[:, 512:1024],
                        lhsT=vH[:, k, :],
                        rhs=a_t[:, 512:1024],
                        start=(k == 0), stop=(k == KC - 1),
                    )
                return emit

            def make_norm(h, qb2, o_t):
                def emit():
                    # den row 64 of o_t = softmax denominators for 1024 q's
                    if psum_recip:
                        # reciprocal straight from PSUM on DVE
                        rec_row = npool.tile([1, 1024], F32, tag="rrow")
                        nc.vector.reciprocal_approx_fast(
                            out=rec_row, in_=o_t[D : D + 1, :]
                        )
                    else:
                        # baseline-style: copy den row to SBUF, recip there
                        den_sb = npool.tile([1, 1024], F32, tag="dsb")
                        nc.scalar.copy(out=den_sb, in_=o_t[D : D + 1, :])
                        rec_row = npool.tile([1, 1024], F32, tag="rrow")
                        nc.vector.reciprocal_approx_fast(
                            out=rec_row, in_=den_sb
                        )
                    bcast = npool.tile([D, 1024], F32, tag="bcast")
                    # replicate rec_row to 64 partitions: step-0 partition AP
                    rec_b = bass.AP(
                        tensor=rec_row.tensor,
                        offset=rec_row.offset,
                        ap=[list(rec_row.ap[0]), [0, D]]
                        + [list(x) for x in rec_row.ap[1:]],
                    )
                    nc.sync.dma_start(out=bcast, in_=rec_b)
                    qs = slice(qb2 * 1024, (qb2 + 1) * 1024)
                    r0 = (h % 2) * 64
                    nc.vector.tensor_mul(
                        out=xt_sb[h // 2][r0 : r0 + 64, qs],
                        in0=o_t[0:D, :],
                        in1=bcast,
                    )
                return emit

            for h in range(HPC):
                vH = v_sb[h]
                for qb2 in range(2):
                    # this unit covers q in [qb2*1024, qb2*1024+1024) of head h
                    qsl = slice(qb2 * 512, (qb2 + 1) * 512)
                    o_full = opool.tile([128, 1024], F32, tag="o_t")
                    o_t = o_full[0 : D + 1, :]
                    pend_att = []
                    for k in range(KC):
                        e_t = epool.tile([128, 1024], F32)
                        # concurrent row-tiled energy pair: T0 q-lo, T8 q-hi
                        # (K duplicated on both partition halves of kt_sb)
                        nc.tensor.matmul(
                            e_t[:, 0:512],
                            lhsT=kt_sb[h][0:64, k * 128 : (k + 1) * 128],
                            rhs=qt_sb[h][0:64, qsl],
                            start=True, stop=True,
                        )
                        nc.tensor.matmul(
                            e_t[:, 512:1024],
                            lhsT=kt_sb[h][64:128, k * 128 : (k + 1) * 128],
                            rhs=qt_sb[h][64:128, qsl],
                            start=True, stop=True,
                        )
                        if k == 1 and pend_norm is not None:
                            pend_norm()
                            pend_norm = None
                        a_t = apool.tile([128, 1024], BF16)
                        if k in dve_groups:
                            nc.vector._custom_dve(
                                exp_op, out=a_t, in0=e_t, s0=B2, s1=B1, imm2=B0
                            )
                        else:
                            nc.scalar.activation(
                                out=a_t,
                                in_=e_t,
                                func=mybir.ActivationFunctionType.Exp,
                                scale=ACT_SCALE,
                            )
                        pend_att.append(make_att(vH, a_t, o_t, k))
                        # drain attV att_batch chunks at a time (fewer PE
                        # tiling-mode switches), keeping exp slack
                        if (
                            k % att_batch == att_batch - 1
                            and len(pend_att) > att_batch
                        ):
                            for _ in range(att_batch):
                                pend_att.pop(0)()
                    for att in pend_att:
                        att()
                    pend_att = []
                    if pend_norm is not None:
                        pend_norm()
                    pend_norm = make_norm(h, qb2, o_t)
            # flush tail
            if pend_norm is not None:
                pend_norm()
                pend_norm = None

            # fc phase: y[q, f] partial, head-pair stacked (contraction 128);
            # both 512-f halves of a q128 tile live in one 2-bank PSUM tile,
            # ordered so consecutive matmuls share the same stationary xt chunk
            for q128 in range(S // 128):
                # reuse the attention-phase PSUM buffers (same tag/size)
                if q128 % 2 == 0:
                    y_t = epool.tile([128, 1024], F32, tag="e_t")
                else:
                    y_t = opool.tile([128, 1024], F32, tag="o_t")
                for p in range(NPAIR):
                    for f in range(2):
                        nc.tensor.matmul(
                            y_t[:, f * 512 : (f + 1) * 512],
                            lhsT=xt_sb[p][:, q128 * 128 : (q128 + 1) * 128],
                            rhs=wt_sb[p][:, f * 512 : (f + 1) * 512],
                            start=(p == 0),
                            stop=(p == NPAIR - 1),
                        )
                y_sb = ysb_pool.tile([128, 1024], F32)
                if q128 % 2 == 0:
                    nc.scalar.copy(out=y_sb, in_=y_t)
                else:
                    nc.vector.tensor_copy(y_sb, y_t)
                nc.sync.dma_start(
                    out=yp[q128 * 128 : (q128 + 1) * 128, :],
                    in_=y_sb,
                )
    nc.compile()
    return nc


class SpmdRunner:
    """Build one jitted shard_map callable over 8 cores; reusable for timing."""

    def __init__(self, nc, n_cores):
        import jax
        from jax.experimental.shard_map import shard_map
        from jax.sharding import Mesh, PartitionSpec

        from concourse import mybir
        from concourse.bass2jax import _bass_exec_p, install_neuronx_cc_hook
        from concourse.bass2jax import partition_id_tensor as _pid

        install_neuronx_cc_hook()
        self.jax = jax
        self.nc = nc
        self.n_cores = n_cores
        self.PartitionSpec = PartitionSpec

        partition_name = nc.partition_id_tensor.name if nc.partition_id_tensor else None
        in_names, out_names, out_avals = [], [], []
        for alloc in nc.m.functions[0].allocations:
            if not isinstance(alloc, mybir.MemoryLocationSet):
                continue
            name = alloc.memorylocations[0].name
            if alloc.kind == "ExternalInput":
                if name != partition_name:
                    in_names.append(name)
            elif alloc.kind == "ExternalOutput":
                out_names.append(name)
                shape = tuple(alloc.tensor_shape)
                dtype = mybir.dt.np(alloc.dtype)
                out_avals.append(jax.core.ShapedArray(shape, dtype))
        self.in_names = in_names
        self.out_names = out_names
        self.out_avals = out_avals
        n_params = len(in_names)
        n_outs = len(out_avals)

        all_in_names = list(in_names) + list(out_names)
        if partition_name is not None:
            all_in_names.append(partition_name)

        def _body(*args):
            operands = list(args)
            if partition_name is not None:
                operands.append(_pid())
            outs = _bass_exec_p.bind(
                *operands,
                out_avals=tuple(out_avals),
                in_names=tuple(all_in_names),
                out_names=tuple(out_names),
                lowering_input_output_aliases=(),
                sim_require_finite=True,
                sim_require_nnan=True,
                nc=nc,
            )
            return tuple(outs)

        self._body = _body
        devices = jax.devices()[:n_cores]
        assert len(devices) == n_cores
        self.mesh = Mesh(np.asarray(devices), ("core",))
        in_specs = (PartitionSpec("core"),) * (n_params + n_outs)
        out_specs = (PartitionSpec("core"),) * n_outs
        self.fn = jax.jit(
            shard_map(
                _body,
                mesh=self.mesh,
                in_specs=in_specs,
                out_specs=out_specs,
                check_rep=False,
            ),
            keep_unused=True,
        )
        self._chain_fns = {}

    def prepare(self, in_maps):
        jax = self.jax
        n = self.n_cores
        concat_in = [
            np.concatenate([np.asarray(in_maps[c][name]) for c in range(n)], axis=0)
            for name in self.in_names
        ]
        concat_zeros = [
            np.zeros((n * a.shape[0], *a.shape[1:]), a.dtype) for a in self.out_avals
        ]
        sharding = jax.sharding.NamedSharding(self.mesh, self.PartitionSpec("core"))
        self.dev_args = [jax.device_put(a, sharding) for a in concat_in + concat_zeros]
        return self.dev_args

    def run(self):
        outs = self.fn(*self.dev_args)
        self.jax.block_until_ready(outs)
        return outs

    def results(self, outs):
        n = self.n_cores
        res = []
        for c in range(n):
            d = {}
            for i, name in enumerate(self.out_names):
                a = np.asarray(outs[i])
                d[name] = a.reshape(n, *self.out_avals[i].shape)[c]
            res.append(d)
        return res

    # ---- timing support: chain K invocations through the tok tensor ----
    def chain_fn(self, k):
        if k in self._chain_fns:
            return self._chain_fns[k]
        jax = self.jax
        from jax.experimental.shard_map import shard_map

        tok_in_idx = self.in_names.index("tok")
        tok_out_idx = self.out_names.index("tok_out")
        n_params = len(self.in_names)

        def _chained(*args):
            args = list(args)
            outs = None
            for _ in range(k):
                outs = self._body(*args)
                args[tok_in_idx] = outs[tok_out_idx]
            return tuple(outs)

        in_specs = (self.PartitionSpec("core"),) * (n_params + len(self.out_names))
        out_specs = (self.PartitionSpec("core"),) * len(self.out_names)
        fn = jax.jit(
            shard_map(
                _chained,
                mesh=self.mesh,
                in_specs=in_specs,
                out_specs=out_specs,
                check_rep=False,
            ),
            keep_unused=True,
        )
        self._chain_fns[k] = fn
        return fn

    def time_chain(self, k, iters=8, warmup=2):
        fn = self.chain_fn(k)
        for _ in range(warmup):
            self.jax.block_until_ready(fn(*self.dev_args))
        ts = []
        for _ in range(iters):
            t0 = time.perf_counter()
            self.jax.block_until_ready(fn(*self.dev_args))
            ts.append(time.perf_counter() - t0)
        return min(ts)


def shard_inputs(values, keys, query, W_out):
    """Build the 8 per-core input maps (host-side layout prep)."""
    v4 = np.asarray(values, np.float32).reshape(NB, S, H, D)
    k4 = np.asarray(keys, np.float32).reshape(NB, S, H, D)
    q4 = np.asarray(query, np.float32).reshape(NB, S, H, D)
    W_out = np.asarray(W_out, np.float32)
    in_maps = []
    tok = np.zeros((1, 128), np.float32)
    for c in range(N_CORES):
        n = c // 4
        h0 = HPC * (c % 4)
        import ml_dtypes

        # per-head Q^T with the two 512-q halves of each 1024-q unit stacked
        # on partition halves: qt[h, 0:64, u*512+j] = Q_h[d, u*1024+j],
        # qt[h, 64:128, u*512+j] = Q_h[d, u*1024+512+j]
        qTh = q4[n, :, h0 : h0 + HPC, :].transpose(1, 2, 0) * np.float32(S_Q)
        # [HPC, 64, S] -> [HPC, 64, 2 units, 2 halves, 512]
        qs5 = qTh.reshape(HPC, D, 2, 2, 512)
        qt = np.ascontiguousarray(
            qs5.transpose(0, 3, 1, 2, 4).reshape(HPC, 128, S // 2)
        ).astype(ml_dtypes.bfloat16)
        # per-head K^T duplicated on both partition halves
        kTh = k4[n, :, h0 : h0 + HPC, :].transpose(1, 2, 0)  # [HPC, 64, S]
        kt = np.ascontiguousarray(
            np.concatenate([kTh, kTh], axis=1)
        ).astype(ml_dtypes.bfloat16)
        vb = np.concatenate(
            [
                np.ascontiguousarray(v4[n, :, h0 : h0 + HPC, :].transpose(1, 0, 2)),
                np.ones((HPC, S, 1), np.float32),
            ],
            axis=2,
        ).astype(ml_dtypes.bfloat16)  # [HPC, S, D+1]
        # W rows for the pair's 128 stacked dims: [NPAIR, 128, E]
        wt = np.ascontiguousarray(
            W_out[:, (h0 * D) : (h0 + HPC) * D].T.reshape(NPAIR, 128, E)
        ).astype(ml_dtypes.bfloat16)
        in_maps.append({"qt": qt, "kt": kt, "vb": vb, "wt": wt, "tok": tok})
    return in_maps


_CACHE = {}


def get_runner():
    if "runner" not in _CACHE:
        nc = build_kernel()
        _CACHE["runner"] = SpmdRunner(nc, N_CORES)
    return _CACHE["runner"]


def kernel(values, keys, query, W_out, b_out):
    runner = get_runner()
    in_maps = shard_inputs(values, keys, query, W_out)
    runner.prepare(in_maps)
    outs = runner.run()
    res = runner.results(outs)
    y = np.zeros((NB, S, E), np.float32)
    for c in range(N_CORES):
        y[c // 4] += res[c]["yp"]
    y += np.asarray(b_out, np.float32)[None, None, :]
    return y


# revision 34
# speedup vs baseline: 1.0281x; 1.0281x over previous
"""Trainium2 Bass kernel for nn_MultiHeadSelfAttention (N=2, S=2048, E=1024, H=16).

Sharding: heads+batch tensor-parallel over 8 cores. Core c handles batch
n = c // 4 and 4 heads h in [4*(c%4), 4*(c%4)+4). Heads are processed in
PAIRS stacked on SBUF partition halves so the PE runs in split (64x128)
row-tiled mode for the whole attention phase:

  energy:  T0 (rows 0-63, head A dims) and T8 (rows 64-127, head B dims)
           execute CONCURRENTLY on the PE array quadrant halves, writing
           separate PSUM banks of one [128, 1024] tile -> ~2x energy rate.
  exp:     ScalarE/VectorE alternate whole [128, 1024] chunks (9:7).
  attV:    per key-chunk, 4 half-contraction matmuls ordered
           [A_T0 || B_T8], [B_T0 || A_T8] so row halves overlap pairwise
           with no same-bank concurrency and no tiling-mode switch; each
           head's o accumulates hi+lo halves sequentially in its own bank.
           The V ones-column yields the softmax denominator as row 64.
  norm:    den row DMA-broadcast to 64 partitions, X = o / den (DVE divide).
  fc_out:  row-parallel, head-pair stacked (contraction 128): y partial
           over this core's 4 heads; host sums 4 partials + bias.
"""

import time

import numpy as np

N_CORES = 8
NB = 2          # batch
S = 2048        # sequence length
E = 1024        # embed size
H = 16          # heads
D = 64          # head dim
HPC = 4         # heads per core
NPAIR = 2       # head pairs per core
SCALE = float(1.0 / np.sqrt(E))  # softmax scale (embed_size**0.5)

KC = S // 128   # 16 contraction chunks of 128 keys
QB = S // 512   # 4 query blocks of 512

# exp(x) ~= p(t)^8, p monic cubic, t = x pre-scaled by S_Q on the host.
# ScalarE chunks use exp(t * ACT_SCALE) (exact); VectorE chunks use the
# 8-stage custom DVE polynomial (rel err ~2.6e-4).
ALPHA = 6.0 ** (1.0 / 3.0)
S_Q = float(SCALE / (8.0 * ALPHA))   # host pre-scale on Q (t = S_Q/SCALE * x)
ACT_SCALE = float(8.0 * ALPHA)
B2, B1, B0 = 1.6574587989430332, 1.8171403999384372, 0.9999891634709047

# groups (key chunks) within a (pair, qb) unit whose exp runs on the DVE
DVE_GROUPS = frozenset({2, 4, 6, 9, 11, 13})


def register_exp_op():
    """Register the EXP_POLY8_ANT custom DVE op (idempotent)."""
    import concourse.dve_ops as dve_ops
    from concourse.dve_ops import OPS, DveOp
    from concourse.dve_spec import C0, C1, C2, Spec, Src0, _has_src1, lower, sq
    from concourse.dve_uop import DveOpSpec

    name = "EXP_POLY8_ANT"
    for op in OPS:
        if op.name == name:
            return op

    _p = ((Src0 + C0) * Src0 + C1) * Src0 + C2
    _body = sq(sq(sq(_p)))

    def _ref(in0, in1, s0, s1, imm2):
        p = ((in0 + s0) * in0 + s1) * in0 + imm2
        return ((p ** 2) ** 2) ** 2

    spec = Spec(body=_body, reference=_ref)
    opcode = dve_ops._CUSTOM_DVE_ROW_BASE + len(OPS)
    shas = {}
    for ver in ("v3", "v4"):
        d = DveOpSpec(
            name=name, opcode=opcode, uops=lower(spec, ver=ver),
            rd1_en=_has_src1(spec),
        )
        shas[ver] = d.sha(ver)
    op = DveOp(name, spec, subdim=False, uops_sha=shas)
    OPS.append(op)
    dve_ops._SUB_OPCODE_FOR_NAME[name] = opcode
    dve_ops.CUSTOM_DVE_SPECS[name] = spec
    return op


def build_kernel(reps=1, dve_groups=DVE_GROUPS, psum_recip=False, att_batch=2):
    import contextlib

    import concourse.bacc as bacc
    import concourse.bass as bass
    import concourse.tile as tile
    from concourse import mybir

    F32 = mybir.dt.float32
    F32R = mybir.dt.float32r
    BF16 = mybir.dt.bfloat16

    exp_op = register_exp_op()

    nc = bacc.Bacc("TRN2", target_bir_lowering=False, num_devices=N_CORES)

    # pair-stacked inputs: [pair, 128 (two heads' dims), S]; bf16 so the
    # row-tiled energy matmuls avoid the fp32 two-pass weight-load path
    qt = nc.dram_tensor("qt", [NPAIR, 128, S], BF16, kind="ExternalInput")
    kt = nc.dram_tensor("kt", [NPAIR, 128, S], BF16, kind="ExternalInput")
    vb = nc.dram_tensor("vb", [HPC, S, D + 1], BF16, kind="ExternalInput")
    wt = nc.dram_tensor("wt", [NPAIR, 128, E], BF16, kind="ExternalInput")
    tok = nc.dram_tensor("tok", [1, 128], F32, kind="ExternalInput")
    yp = nc.dram_tensor("yp", [S, E], F32, kind="ExternalOutput")
    tok_out = nc.dram_tensor("tok_out", [1, 128], F32, kind="ExternalOutput")

    with tile.TileContext(nc) as tc:
        with contextlib.ExitStack() as ctx:
            singles = ctx.enter_context(tc.tile_pool(name="singles", bufs=1))
            vpool = ctx.enter_context(tc.tile_pool(name="vpool", bufs=4))
            epool = ctx.enter_context(
                tc.tile_pool(name="epool", bufs=2, space="PSUM")
            )
            opool = ctx.enter_context(
                tc.tile_pool(name="opool", bufs=2, space="PSUM")
            )
            apool = ctx.enter_context(tc.tile_pool(name="apool", bufs=5))
            npool = ctx.enter_context(tc.tile_pool(name="npool", bufs=2))
            ysb_pool = ctx.enter_context(tc.tile_pool(name="ysb", bufs=4))

            # token passthrough for timing chains
            tok_sb = singles.tile([1, 128], F32)
            nc.gpsimd.dma_start(out=tok_sb, in_=tok[:, :])
            nc.gpsimd.dma_start(out=tok_out[:, :], in_=tok_sb)

            # resident inputs (outside the reps loop)
            qt_sb, kt_sb, wt_sb, xt_sb = [], [], [], []
            for p in range(NPAIR):
                q_t = singles.tile([128, S], BF16, tag=f"qt{p}")
                nc.sync.dma_start(out=q_t, in_=qt[p])
                qt_sb.append(q_t)
                k_t = singles.tile([128, S], BF16, tag=f"kt{p}")
                nc.sync.dma_start(out=k_t, in_=kt[p])
                kt_sb.append(k_t)
                w_t = singles.tile([128, E], BF16, tag=f"wt{p}")
                nc.sync.dma_start(out=w_t, in_=wt[p])
                wt_sb.append(w_t)
                x_t = singles.tile([128, S], BF16, tag=f"xt{p}")
                xt_sb.append(x_t)

            loop_cm = tc.For_i(0, reps, 1) if reps > 1 else contextlib.nullcontext()
            ctx.enter_context(loop_cm)

            # V tiles: [128 keys-in-chunk, kc, 65], one per head, prefetched
            v_sb = []
            for h in range(HPC):
                v_t = vpool.tile([128, KC, D + 1], BF16, tag=f"v{h}")
                nc.sync.dma_start(
                    out=v_t, in_=vb[h].rearrange("(kc p) c -> p kc c", p=128)
                )
                v_sb.append(v_t)

            pend_att = None   # deferred attV emission (one group behind)
            pend_norm = None  # deferred normalization (one unit behind)

            def make_att(vA, vB, a_t, o_t, k):
                def emit():
                    # full-row (128-contraction) attV: serializes against both
                    # row chains, so no PSUM bank can see concurrent writes
                    nc.tensor.matmul(
                        o_t[:, 0:512],
                        lhsT=vA[:, k, :],
                        rhs=a_t[:, 0:512],
                        start=(k == 0), stop=(k == KC - 1),
                    )
                    nc.tensor.matmul(
                        o_t[:, 512:1024],
                        lhsT=vB[:, k, :],
                        rhs=a_t[:, 512:1024],
                        start=(k == 0), stop=(k == KC - 1),
                    )
                return emit

            def make_norm(p, qb, o_t):
                def emit():
                    # den row 64 of o_t = [den_A | den_B]
                    if psum_recip:
                        # reciprocal straight from PSUM on DVE
                        rec_row = npool.tile([1, 1024], F32, tag="rrow")
                        nc.vector.reciprocal_approx_fast(
                            out=rec_row, in_=o_t[D : D + 1, :]
                        )
                    else:
                        # baseline-style: copy den row to SBUF, recip there
                        den_sb = npool.tile([1, 1024], F32, tag="dsb")
                        nc.scalar.copy(out=den_sb, in_=o_t[D : D + 1, :])
                        rec_row = npool.tile([1, 1024], F32, tag="rrow")
                        nc.vector.reciprocal_approx_fast(
                            out=rec_row, in_=den_sb
                        )
                    bcast = npool.tile([D, 1024], F32, tag="bcast")
                    # replicate rec_row to 64 partitions: step-0 partition AP
                    rec_b = bass.AP(
                        tensor=rec_row.tensor,
                        offset=rec_row.offset,
                        ap=[list(rec_row.ap[0]), [0, D]]
                        + [list(x) for x in rec_row.ap[1:]],
                    )
                    nc.sync.dma_start(out=bcast, in_=rec_b)
                    qs = slice(qb * 512, (qb + 1) * 512)
                    nc.vector.tensor_mul(
                        out=xt_sb[p][0:64, qs],
                        in0=o_t[0:D, 0:512],
                        in1=bcast[:, 0:512],
                    )
                    nc.vector.tensor_mul(
                        out=xt_sb[p][64:128, qs],
                        in0=o_t[0:D, 512:1024],
                        in1=bcast[:, 512:1024],
                    )
                return emit

            for p in range(NPAIR):
                vA, vB = v_sb[2 * p], v_sb[2 * p + 1]
                for qb in range(QB):
                    qsl = slice(qb * 512, (qb + 1) * 512)
                    o_full = opool.tile([128, 1024], F32, tag="o_t")
                    o_t = o_full[0 : D + 1, :]
                    pend_att = []
                    for k in range(KC):
                        e_t = epool.tile([128, 1024], F32)
                        # concurrent row-tiled energy pair: T0 head A, T8 head B
                        nc.tensor.matmul(
                            e_t[:, 0:512],
                            lhsT=kt_sb[p][0:64, k * 128 : (k + 1) * 128],
                            rhs=qt_sb[p][0:64, qsl],
                            start=True, stop=True,
                        )
                        nc.tensor.matmul(
                            e_t[:, 512:1024],
                            lhsT=kt_sb[p][64:128, k * 128 : (k + 1) * 128],
                            rhs=qt_sb[p][64:128, qsl],
                            start=True, stop=True,
                        )
                        if k == 1 and pend_norm is not None:
                            pend_norm()
                            pend_norm = None
                        a_t = apool.tile([128, 1024], BF16)
                        if k in dve_groups:
                            nc.vector._custom_dve(
                                exp_op, out=a_t, in0=e_t, s0=B2, s1=B1, imm2=B0
                            )
                        else:
                            nc.scalar.activation(
                                out=a_t,
                                in_=e_t,
                                func=mybir.ActivationFunctionType.Exp,
                                scale=ACT_SCALE,
                            )
                        pend_att.append(make_att(vA, vB, a_t, o_t, k))
                        # drain attV att_batch chunks at a time (fewer PE
                        # tiling-mode switches), keeping exp slack
                        if (
                            k % att_batch == att_batch - 1
                            and len(pend_att) > att_batch
                        ):
                            for _ in range(att_batch):
                                pend_att.pop(0)()
                    for att in pend_att:
                        att()
                    pend_att = []
                    if pend_norm is not None:
                        pend_norm()
                    pend_norm = make_norm(p, qb, o_t)
            # flush tail
            if pend_norm is not None:
                pend_norm()
                pend_norm = None

            # fc phase: y[q, f] partial, head-pair stacked (contraction 128)
            for q128 in range(S // 128):
                for f in range(E // 512):
                    # reuse the attention-phase PSUM buffers (same tag/size)
                    if (q128 * 2 + f) % 2 == 0:
                        y_full = epool.tile([128, 1024], F32, tag="e_t")
                    else:
                        y_full = opool.tile([128, 1024], F32, tag="o_t")
                    y_t = y_full[:, 0:512]
                    for p in range(NPAIR):
                        nc.tensor.matmul(
                            y_t,
                            lhsT=xt_sb[p][:, q128 * 128 : (q128 + 1) * 128],
                            rhs=wt_sb[p][:, f * 512 : (f + 1) * 512],
                            start=(p == 0),
                            stop=(p == NPAIR - 1),
                        )
                    y_sb = ysb_pool.tile([128, 512], F32)
                    if (q128 * 2 + f) % 2 == 0:
                        nc.scalar.copy(out=y_sb, in_=y_t)
                    else:
                        nc.vector.tensor_copy(y_sb, y_t)
                    nc.sync.dma_start(
                        out=yp[
                            q128 * 128 : (q128 + 1) * 128, f * 512 : (f + 1) * 512
                        ],
                        in_=y_sb,
                    )
    nc.compile()
    return nc


class SpmdRunner:
    """Build one jitted shard_map callable over 8 cores; reusable for timing."""

    def __init__(self, nc, n_cores):
        import jax
        from jax.experimental.shard_map import shard_map
        from jax.sharding import Mesh, PartitionSpec

        from concourse import mybir
        from concourse.bass2jax import _bass_exec_p, install_neuronx_cc_hook
        from concourse.bass2jax import partition_id_tensor as _pid

        install_neuronx_cc_hook()
        self.jax = jax
        self.nc = nc
        self.n_cores = n_cores
        self.PartitionSpec = PartitionSpec

        partition_name = nc.partition_id_tensor.name if nc.partition_id_tensor else None
        in_names, out_names, out_avals = [], [], []
        for alloc in nc.m.functions[0].allocations:
            if not isinstance(alloc, mybir.MemoryLocationSet):
                continue
            name = alloc.memorylocations[0].name
            if alloc.kind == "ExternalInput":
                if name != partition_name:
                    in_names.append(name)
            elif alloc.kind == "ExternalOutput":
                out_names.append(name)
                shape = tuple(alloc.tensor_shape)
                dtype = mybir.dt.np(alloc.dtype)
                out_avals.append(jax.core.ShapedArray(shape, dtype))
        self.in_names = in_names
        self.out_names = out_names
        self.out_avals = out_avals
        n_params = len(in_names)
        n_outs = len(out_avals)

        all_in_names = list(in_names) + list(out_names)
        if partition_name is not None:
            all_in_names.append(partition_name)

        def _body(*args):
            operands = list(args)
            if partition_name is not None:
                operands.append(_pid())
            outs = _bass_exec_p.bind(
                *operands,
                out_avals=tuple(out_avals),
                in_names=tuple(all_in_names),
                out_names=tuple(out_names),
                lowering_input_output_aliases=(),
                sim_require_finite=True,
                sim_require_nnan=True,
                nc=nc,
            )
            return tuple(outs)

        self._body = _body
        devices = jax.devices()[:n_cores]
        assert len(devices) == n_cores
        self.mesh = Mesh(np.asarray(devices), ("core",))
        in_specs = (PartitionSpec("core"),) * (n_params + n_outs)
        out_specs = (PartitionSpec("core"),) * n_outs
        self.fn = jax.jit(
            shard_map(
                _body,
                mesh=self.mesh,
                in_specs=in_specs,
                out_specs=out_specs,
                check_rep=False,
            ),
            keep_unused=True,
        )
        self._chain_fns = {}

    def prepare(self, in_maps):
        jax = self.jax
        n = self.n_cores
        concat_in = [
            np.concatenate([np.asarray(in_maps[c][name]) for c in range(n)], axis=0)
            for name in self.in_names
        ]
        concat_zeros = [
            np.zeros((n * a.shape[0], *a.shape[1:]), a.dtype) for a in self.out_avals
        ]
        sharding = jax.sharding.NamedSharding(self.mesh, self.PartitionSpec("core"))
        self.dev_args = [jax.device_put(a, sharding) for a in concat_in + concat_zeros]
        return self.dev_args

    def run(self):
        outs = self.fn(*self.dev_args)
        self.jax.block_until_ready(outs)
        return outs

    def results(self, outs):
        n = self.n_cores
        res = []
        for c in range(n):
            d = {}
            for i, name in enumerate(self.out_names):
                a = np.asarray(outs[i])
                d[name] = a.reshape(n, *self.out_avals[i].shape)[c]
            res.append(d)
        return res

    # ---- timing support: chain K invocations through the tok tensor ----
    def chain_fn(self, k):
        if k in self._chain_fns:
            return self._chain_fns[k]
        jax = self.jax
        from jax.experimental.shard_map import shard_map

        tok_in_idx = self.in_names.index("tok")
        tok_out_idx = self.out_names.index("tok_out")
        n_params = len(self.in_names)

        def _chained(*args):
            args = list(args)
            outs = None
            for _ in range(k):
                outs = self._body(*args)
                args[tok_in_idx] = outs[tok_out_idx]
            return tuple(outs)

        in_specs = (self.PartitionSpec("core"),) * (n_params + len(self.out_names))
        out_specs = (self.PartitionSpec("core"),) * len(self.out_names)
        fn = jax.jit(
            shard_map(
                _chained,
                mesh=self.mesh,
                in_specs=in_specs,
                out_specs=out_specs,
                check_rep=False,
            ),
            keep_unused=True,
        )
        self._chain_fns[k] = fn
        return fn

    def time_chain(self, k, iters=8, warmup=2):
        fn = self.chain_fn(k)
        for _ in range(warmup):
            self.jax.block_until_ready(fn(*self.dev_args))
        ts = []
        for _ in range(iters):
            t0 = time.perf_counter()
            self.jax.block_until_ready(fn(*self.dev_args))
            ts.append(time.perf_counter() - t0)
        return min(ts)


def shard_inputs(values, keys, query, W_out):
    """Build the 8 per-core input maps (host-side layout prep)."""
    v4 = np.asarray(values, np.float32).reshape(NB, S, H, D)
    k4 = np.asarray(keys, np.float32).reshape(NB, S, H, D)
    q4 = np.asarray(query, np.float32).reshape(NB, S, H, D)
    W_out = np.asarray(W_out, np.float32)
    in_maps = []
    tok = np.zeros((1, 128), np.float32)
    for c in range(N_CORES):
        n = c // 4
        h0 = HPC * (c % 4)
        import ml_dtypes

        # pair-stacked Q^T/K^T: [NPAIR, 128, S]; rows 0-63 head 2p, 64-127 head 2p+1
        qt = np.ascontiguousarray(
            q4[n, :, h0 : h0 + HPC, :].transpose(1, 2, 0) * np.float32(S_Q)
        ).reshape(NPAIR, 128, S).astype(ml_dtypes.bfloat16)
        kt = np.ascontiguousarray(
            k4[n, :, h0 : h0 + HPC, :].transpose(1, 2, 0)
        ).reshape(NPAIR, 128, S).astype(ml_dtypes.bfloat16)
        vb = np.concatenate(
            [
                np.ascontiguousarray(v4[n, :, h0 : h0 + HPC, :].transpose(1, 0, 2)),
                np.ones((HPC, S, 1), np.float32),
            ],
            axis=2,
        ).astype(ml_dtypes.bfloat16)  # [HPC, S, D+1]
        # W rows for the pair's 128 stacked dims: [NPAIR, 128, E]
        wt = np.ascontiguousarray(
            W_out[:, (h0 * D) : (h0 + HPC) * D].T.reshape(NPAIR, 128, E)
        ).astype(ml_dtypes.bfloat16)
        in_maps.append({"qt": qt, "kt": kt, "vb": vb, "wt": wt, "tok": tok})
    return in_maps


_CACHE = {}


def get_runner():
    if "runner" not in _CACHE:
        nc = build_kernel()
        _CACHE["runner"] = SpmdRunner(nc, N_CORES)
    return _CACHE["runner"]


def kernel(values, keys, query, W_out, b_out):
    runner = get_runner()
    in_maps = shard_inputs(values, keys, query, W_out)
    runner.prepare(in_maps)
    outs = runner.run()
    res = runner.results(outs)
    y = np.zeros((NB, S, E), np.float32)
    for c in range(N_CORES):
        y[c // 4] += res[c]["yp"]
    y += np.asarray(b_out, np.float32)[None, None, :]
    return y
